# revision 6
# baseline (speedup 1.0000x reference)
"""Trainium2 Bass kernel for nn_MultiHeadAttention_88210038326473.

Reference computation (B=4, S=2048, HID=2048, H=16, DH=128):
    Q = queries @ Wq.T + bq ; K = keys @ Wk.T + bk ; V = keys @ Wv.T + bv
    per-head scores = Qh Kh^T / sqrt(HID), key-padding + causal mask,
    softmax, out = attn @ Vh, concat heads, + queries residual.

Sharding: 8 cores = 4 batches x 2 head-groups (8 heads each). Each core
computes out[b, :, hg*1024:(hg+1)*1024] (stored transposed [1024, 2048];
host transposes back and assembles).

Device algorithm per core (everything SBUF-resident, no DRAM scratch):
  Phase KV: KT[e,s] = (keys @ Wk.T).T and V[s,e] = keys @ Wv.T + bv,
            written straight into resident SBUF tiles (8x[128,2048] KT,
            16x[128,1024] V).
  Phase Q:  QT[e,s] = (queries @ Wq.T).T -> resident SBUF (8x[128,2048]).
  Attention per (head, q-chunk of 512): transposed scores sT[k,q] =
            KT_h^T QT_h per 128-k-tile (causal tiles only). Full k-tiles
            go to 2-bank PSUM strips exp'd with ONE activation per strip;
            the 4 diagonal-band tiles are packed contiguously (widths
            512/384/128 in one 2-bank strip + 256 in another) so two
            activations cover them with no wasted exp columns. No max
            subtraction (scores are O(1)). Diagonal 128x128 blocks masked
            by a 0/1 triangle multiply (DVE). V-matmul accumulates
            outT[d,q] += V_kt^T expT; row sums accumulate BROADCAST via an
            all-ones [128,128] stationary matmul into [128,512] PSUM (so
            no [1,N] ops anywhere); reciprocal_approx_fast (DVE, full
            width), normalize + bf16 residual add.
"""

import contextlib
import math

import numpy as np

B, S, HID, H, DH = 4, 2048, 2048, 16, 128
NCORES = 8
HPC = 8          # heads per core
EH = HPC * DH    # 1024 e-dims per core
SCALE = 1.0 / math.sqrt(HID)
QC = 512         # attention q-chunk
NQC = S // QC    # 4
NKT = S // DH    # 16 k-tiles
NF = HID // DH   # 16 f-tiles (contraction)
PC = 512         # projection s-chunk (matmul moving N)
NPC = S // PC    # 4
NST = S // DH    # 16 V s-tiles
NEG_BIAS = np.float32(-1.0e30)
ACT_DT = "bf16"
COMPUTE_MAX_WAITS = 1

# band packing: (j, pos, width) -- band tile j covers q-cols [j*128, 512) of
# the q-chunk, packed contiguously at `pos` so one exp covers a whole strip
# and no matmul output straddles a PSUM bank boundary.
PACK_A = ((0, 0, 512), (1, 512, 384), (3, 896, 128))   # strip A: 2 banks
PACK_B = ((2, 0, 256),)                                # strip B

CTRL_OPS = ("InstDrain", "InstNoOp", "InstEventSemaphore", "InstISA")


def _split_excess_waits(nc, max_waits=1, compute_max_waits=None):
    """walrus in this container rejects >1 sem-wait per CTRL-class instruction.
    Move excess waits onto preceding NoOps on the same engine."""
    import concourse.mybir as mybir

    if compute_max_waits is None:
        compute_max_waits = max_waits
    n_split = 0
    for fn in nc.m.functions:
        for blk in fn.blocks:
            insts = list(blk.instructions)
            out = []
            changed = False
            for ins in insts:
                lim = (
                    max_waits
                    if type(ins).__name__ in CTRL_OPS
                    else compute_max_waits
                )
                si = ins.sync_info
                if si is not None and si.on_wait and len(si.on_wait) > lim:
                    waits = list(si.on_wait)
                    carriers, rest = waits[:-lim], waits[-lim:]
                    for i in range(0, len(carriers), max_waits):
                        chunk = carriers[i : i + max_waits]
                        out.append(
                            mybir.InstNoOp(
                                name=f"{ins.name}-ws{i}",
                                engine=ins.engine,
                                bass_nofuse=True,
                                sync_info=mybir.SyncInfo(on_wait=chunk, on_update=[]),
                            )
                        )
                        n_split += 1
                    ins.sync_info = mybir.SyncInfo(
                        on_wait=rest, on_update=list(si.on_update)
                    )
                    changed = True
                out.append(ins)
            if changed:
                blk.instructions = out
    return n_split


_CACHE = {}


def _build(fast=True, phases=("k", "q", "attn"), reps=1, act_dt="bf16",
           scale=None):
    """Build the (core-uniform) Bass program. Returns nc.

    fast=True drops the key-padding bias from the exp (valid when no key is
    padding -- checked on host) and enables strip-consolidated exps.
    fast=False applies the per-k-tile padding bias (general path, per-tile
    exps)."""
    key = ("nc", fast, tuple(phases), reps, act_dt)
    if key in _CACHE:
        return _CACHE[key]

    import concourse.bass as bass
    import concourse.mybir as mybir
    from concourse.tile import TileContext

    F32 = mybir.dt.float32
    ADT = mybir.dt.bfloat16 if act_dt == "bf16" else mybir.dt.float32r
    EXP = mybir.ActivationFunctionType.Exp
    LN = mybir.ActivationFunctionType.Ln
    IDENT = mybir.ActivationFunctionType.Identity

    nc = bass.Bass("TRN2", target_bir_lowering=False, debug=False)

    qT = nc.dram_tensor("qT", [HID, S], ADT, kind="ExternalInput")
    kT = nc.dram_tensor("kT", [HID, S], ADT, kind="ExternalInput")
    wqT = nc.dram_tensor("wqT", [HID, EH], ADT, kind="ExternalInput")
    wkT = nc.dram_tensor("wkT", [HID, EH], ADT, kind="ExternalInput")
    wvT = nc.dram_tensor("wvT", [HID, EH], ADT, kind="ExternalInput")
    bq_d = nc.dram_tensor("bq_d", [DH, HPC], F32, kind="ExternalInput")
    bk_d = nc.dram_tensor("bk_d", [DH, HPC], F32, kind="ExternalInput")
    bv_d = nc.dram_tensor("bv_d", [1, EH], ADT, kind="ExternalInput")
    kbias_d = nc.dram_tensor("kbias_d", [DH, NKT], F32, kind="ExternalInput")
    tri_d = nc.dram_tensor("tri_d", [DH, DH], ADT, kind="ExternalInput")
    ones_sq_d = nc.dram_tensor("ones_sq_d", [DH, DH], ADT, kind="ExternalInput")
    ones_ra_d = nc.dram_tensor("ones_ra_d", [1, DH], ADT, kind="ExternalInput")
    resid_d = nc.dram_tensor("resid_d", [EH, S], ADT, kind="ExternalInput")
    outT_d = nc.dram_tensor("outT_d", [EH, S], F32, kind="ExternalOutput")

    qT3 = qT[:].rearrange("(f p) s -> p f s", p=DH)
    kT3 = kT[:].rearrange("(f p) s -> p f s", p=DH)
    wq3 = wqT[:].rearrange("(f p) e -> p f e", p=DH)
    wk3 = wkT[:].rearrange("(f p) e -> p f e", p=DH)
    wv3 = wvT[:].rearrange("(f p) e -> p f e", p=DH)

    with TileContext(nc) as tc, nc.allow_low_precision(reason="bf16 ~ fp32"):
        with tc.tile_pool(name="persist", bufs=1) as persist:
            tri = persist.tile([DH, DH], ADT, tag="tri")
            kbias = persist.tile([DH, NKT], F32, tag="kbias")
            ones_sq = persist.tile([DH, DH], ADT, tag="ones_sq")
            ones_ra = persist.tile([1, DH], ADT, tag="ones_ra")
            bq_sb = persist.tile([DH, HPC], F32, tag="bq")
            bk_sb = persist.tile([DH, HPC], F32, tag="bk")
            bv_sb = persist.tile([1, EH], ADT, tag="bv")
            nc.sync.dma_start(tri[:], tri_d[:])
            nc.sync.dma_start(kbias[:], kbias_d[:])
            nc.sync.dma_start(ones_sq[:], ones_sq_d[:])
            nc.sync.dma_start(ones_ra[:], ones_ra_d[:])
            nc.sync.dma_start(bq_sb[:], bq_d[:])
            nc.sync.dma_start(bk_sb[:], bk_d[:])
            nc.sync.dma_start(bv_sb[:], bv_d[:])

            for _rep in range(reps):
                with contextlib.ExitStack() as repstack:
                    _rep_body(
                        nc, tc, repstack, phases, fast,
                        kT3, qT3, wk3, wv3, wq3, resid_d, outT_d,
                        tri, kbias, ones_sq, ones_ra, bq_sb, bk_sb, bv_sb,
                        F32, ADT, EXP, LN, IDENT,
                    )

    _split_excess_waits(nc, max_waits=1, compute_max_waits=COMPUTE_MAX_WAITS)
    _CACHE[key] = nc
    return nc


def _rep_body(
    nc, tc, repstack, phases, fast,
    kT3, qT3, wk3, wv3, wq3, resid_d, outT_d,
    tri, kbias, ones_sq, ones_ra, bq_sb, bk_sb, bv_sb,
    F32, ADT, EXP, LN, IDENT,
):
    ktvp = repstack.enter_context(tc.tile_pool(name="ktv", bufs=1))
    kt_r = [ktvp.tile([DH, S], ADT, tag=f"ktr{et}", name=f"ktr{et}")
            for et in range(HPC)]
    v_r = [ktvp.tile([DH, EH], ADT, tag=f"vr{st}", name=f"vr{st}")
           for st in range(NST)]
    wqp = repstack.enter_context(tc.tile_pool(name="wq", bufs=1))
    wq_t = wqp.tile([DH, NF * EH], ADT, tag="wq", name="wq")

    # ---------------- Phase KV ----------------
    if "k" in phases:
        with tc.tile_pool(name="wk", bufs=1) as wkp, \
             tc.tile_pool(name="wv", bufs=1) as wvp, \
             tc.tile_pool(name="kc", bufs=2) as kcp, \
             tc.tile_pool(name="pk", bufs=3, space="PSUM") as pkp, \
             tc.tile_pool(name="pv", bufs=3, space="PSUM") as pvp:
            wk_t = wkp.tile([DH, NF * EH], ADT, tag="wk", name="wk")
            wv_t = wvp.tile([DH, NF * EH], ADT, tag="wv", name="wv")
            # weight loads on the sync DGE queue; kc chunks go on the
            # scalar queue so they stream concurrently.
            nc.sync.dma_start(wk_t[:].rearrange("p (f e) -> p f e", f=NF), wk3)
            nc.sync.dma_start(wv_t[:].rearrange("p (f e) -> p f e", f=NF), wv3)
            nc.sync.dma_start(wq_t[:].rearrange("p (f e) -> p f e", f=NF), wq3)
            for sc in range(NPC):
                s0 = sc * PC
                kc = kcp.tile([DH, NF * PC], ADT, tag="kc", name="kc")
                nc.scalar.dma_start(
                    kc[:].rearrange("p (f s) -> p f s", f=NF),
                    kT3[:, :, s0 : s0 + PC],
                )
                for et in range(HPC):
                    pk = pkp.tile([DH, PC], F32, name="pk")
                    for f in range(NF):
                        nc.tensor.matmul(
                            pk[:],
                            wk_t[:, f * EH + et * DH : f * EH + (et + 1) * DH],
                            kc[:, f * PC : (f + 1) * PC],
                            start=(f == 0),
                            stop=(f == NF - 1),
                        )
                    nc.scalar.activation(
                        kt_r[et][:, s0 : s0 + PC], pk[:], IDENT,
                        bias=bk_sb[:, et : et + 1],
                    )
                for sti in range(PC // DH):
                    st = sc * (PC // DH) + sti
                    for ec in range(EH // QC):
                        pv = pvp.tile([DH, QC], F32, name="pv")
                        for f in range(NF):
                            nc.tensor.matmul(
                                pv[:],
                                kc[:, f * PC + sti * DH : f * PC + (sti + 1) * DH],
                                wv_t[:, f * EH + ec * QC : f * EH + (ec + 1) * QC],
                                start=(f == 0),
                                stop=False,
                            )
                        nc.tensor.matmul(
                            pv[:],
                            ones_ra[:],
                            bv_sb[:, ec * QC : (ec + 1) * QC],
                            start=False,
                            stop=True,
                        )
                        nc.scalar.copy(v_r[st][:, ec * QC : (ec + 1) * QC], pv[:])

    # ---------------- Phase Q (QT resident) ----------------
    qtp = repstack.enter_context(tc.tile_pool(name="qt", bufs=1))
    qt_r = [qtp.tile([DH, S], ADT, tag=f"qt{et}", name=f"qt{et}")
            for et in range(HPC)]
    if "q" in phases:
        with tc.tile_pool(name="qc", bufs=2) as qcp, \
             tc.tile_pool(name="pq", bufs=4, space="PSUM") as pqp:
            for sc in range(NPC):
                s0 = sc * PC
                qch = qcp.tile([DH, NF * PC], ADT, tag="qch", name="qch")
                nc.scalar.dma_start(
                    qch[:].rearrange("p (f s) -> p f s", f=NF),
                    qT3[:, :, s0 : s0 + PC],
                )
                for et in range(HPC):
                    pq = pqp.tile([DH, PC], F32, name="pq")
                    for f in range(NF):
                        nc.tensor.matmul(
                            pq[:],
                            wq_t[:, f * EH + et * DH : f * EH + (et + 1) * DH],
                            qch[:, f * PC : (f + 1) * PC],
                            start=(f == 0),
                            stop=(f == NF - 1),
                        )
                    nc.scalar.activation(
                        qt_r[et][:, s0 : s0 + PC], pq[:], IDENT,
                        bias=bq_sb[:, et : et + 1],
                    )

    # ---------------- Phase attention ----------------
    if "attn" in phases:
        _attention(
            nc, tc, fast, kt_r, v_r, qt_r, resid_d, outT_d,
            tri, kbias, ones_sq, F32, ADT, EXP, LN,
        )


def _attention(
    nc, tc, fast, kt_r, v_r, qt_r, resid_d, outT_d,
    tri, kbias, ones_sq, F32, ADT, EXP, LN,
):
    with tc.tile_pool(name="ex", bufs=4) as exp_p, \
         tc.tile_pool(name="rsd", bufs=2) as rsdp, \
         tc.tile_pool(name="outs", bufs=2) as outp, \
         tc.tile_pool(name="tail", bufs=2) as tailp, \
         tc.tile_pool(name="ps_s", bufs=2, space="PSUM") as pss, \
         tc.tile_pool(name="ps_o", bufs=2, space="PSUM") as pso, \
         tc.tile_pool(name="ps_r", bufs=2, space="PSUM") as psr:
        for h in range(HPC):
            rsd = rsdp.tile([DH, S], ADT, tag="rsd", name="rsd")
            nc.sync.dma_start(rsd[:], resid_d[h * DH : (h + 1) * DH, :])
            oth = outp.tile([DH, S], F32, tag="oth", name="oth")
            ktH = kt_r[h]
            qtH = qt_r[h]
            for qc in range(NQC):
                q0 = qc * QC
                nfull = 4 * qc
                nkt = nfull + 4
                po = pso.tile([DH, QC], F32, name="po")
                rs = psr.tile([DH, QC], F32, name="rs")
                n_emitted = [0]

                def consume(kt, exs, off, width):
                    first = n_emitted[0] == 0
                    last = n_emitted[0] == nkt - 1
                    n_emitted[0] += 1
                    nc.tensor.matmul(
                        po[:, off : off + width],
                        v_r[kt][:, h * DH : (h + 1) * DH],
                        exs,
                        start=first,
                        stop=last,
                    )
                    nc.tensor.matmul(
                        rs[:, off : off + width],
                        ones_sq[:],
                        exs,
                        start=first,
                        stop=last,
                    )

                # full k-tiles in 2-bank strips of 2
                for s2 in range(nfull // 2):
                    ps = pss.tile([DH, 2 * QC], F32, tag="ps", name="ps")
                    for u in (0, 1):
                        kt = 2 * s2 + u
                        nc.tensor.matmul(
                            ps[:, u * QC : (u + 1) * QC],
                            ktH[:, kt * DH : (kt + 1) * DH],
                            qtH[:, q0 : q0 + QC],
                            start=True,
                            stop=True,
                        )
                    ex = exp_p.tile([DH, 2 * QC], ADT, tag="ex", name="ex")
                    if fast:
                        nc.scalar.activation(ex[:], ps[:], EXP, scale=float(SCALE))
                    else:
                        for u in (0, 1):
                            kt = 2 * s2 + u
                            nc.scalar.activation(
                                ex[:, u * QC : (u + 1) * QC],
                                ps[:, u * QC : (u + 1) * QC],
                                EXP, bias=kbias[:, kt : kt + 1],
                                scale=float(SCALE),
                            )
                    for u in (0, 1):
                        consume(2 * s2 + u, ex[:, u * QC : (u + 1) * QC], 0, QC)

                # diagonal band: packed strips
                band_ex = {}
                for pack in (PACK_A, PACK_B):
                    ps = pss.tile([DH, 2 * QC], F32, tag="ps", name="psb")
                    exb = exp_p.tile([DH, 2 * QC], ADT, tag="ex", name="exb")
                    tot = max(pos + w for (_, pos, w) in pack)
                    for (j, pos, w) in pack:
                        kt = nfull + j
                        off = j * DH
                        nc.tensor.matmul(
                            ps[:, pos : pos + w],
                            ktH[:, kt * DH : (kt + 1) * DH],
                            qtH[:, q0 + off : q0 + QC],
                            start=True,
                            stop=True,
                        )
                    if fast:
                        nc.scalar.activation(
                            exb[:, 0:tot], ps[:, 0:tot], EXP, scale=float(SCALE)
                        )
                    else:
                        for (j, pos, w) in pack:
                            kt = nfull + j
                            nc.scalar.activation(
                                exb[:, pos : pos + w], ps[:, pos : pos + w],
                                EXP, bias=kbias[:, kt : kt + 1],
                                scale=float(SCALE),
                            )
                    for (j, pos, w) in pack:
                        # causal triangle on the leading 128x128 diag block
                        nc.vector.tensor_mul(
                            exb[:, pos : pos + DH], exb[:, pos : pos + DH], tri[:]
                        )
                        band_ex[j] = exb[:, pos : pos + w]
                # consume bands in fixed order (j=2 last carries stop=True)
                for j in (0, 1, 3, 2):
                    consume(nfull + j, band_ex[j], j * DH, QC - j * DH)

                # tail: broadcasted row-sums -> 1/sum = exp(-ln(sum)) on ACT
                # (ln+exp live in one table set with the attention exps, so
                # no ACT_TABLE_LOAD thrash) -> normalize + resid on DVE.
                lns = tailp.tile([DH, QC], F32, tag="lns", name="lns")
                nc.scalar.activation(lns[:], rs[:], LN)
                rec = tailp.tile([DH, QC], F32, tag="rec", name="rec")
                nc.scalar.activation(rec[:], lns[:], EXP, scale=-1.0)
                nc.vector.tensor_mul(oth[:, q0 : q0 + QC], po[:], rec[:])
                nc.vector.tensor_add(
                    oth[:, q0 : q0 + QC],
                    oth[:, q0 : q0 + QC],
                    rsd[:, q0 : q0 + QC],
                )
            nc.sync.dma_start(outT_d[h * DH : (h + 1) * DH, :], oth[:])


def _host_prep(queries, keys, Wq, bq, Wk, bk, Wv, bv, act_dt=None):
    """Build the 8 per-core input maps (host-side shard + layout prep)."""
    if act_dt is None:
        act_dt = ACT_DT
    if act_dt == "bf16":
        import ml_dtypes

        adt = ml_dtypes.bfloat16
    else:
        adt = np.float32
    queries = np.ascontiguousarray(queries, dtype=np.float32)
    keys = np.ascontiguousarray(keys, dtype=np.float32)

    qTa = np.ascontiguousarray(queries.transpose(0, 2, 1)).astype(adt)  # [B,HID,S]
    kTa = np.ascontiguousarray(keys.transpose(0, 2, 1)).astype(adt)
    WqT = np.ascontiguousarray(np.asarray(Wq, np.float32).T).astype(adt)  # [f, e]
    WkT = np.ascontiguousarray(np.asarray(Wk, np.float32).T).astype(adt)
    WvT = np.ascontiguousarray(np.asarray(Wv, np.float32).T).astype(adt)
    bq = np.asarray(bq, np.float32)
    bk = np.asarray(bk, np.float32)
    bv = np.asarray(bv, np.float32)

    # key padding mask -> additive bias per (b, k): 0 keep, -1e30 mask
    ksum = keys.sum(axis=-1)  # [B, S]
    kbias_all = np.where(ksum != 0.0, np.float32(0), NEG_BIAS).astype(np.float32)

    # causal triangle for the diagonal 128x128 blocks: keep iff q_local >= k_local
    tri = (np.arange(DH)[None, :] >= np.arange(DH)[:, None]).astype(adt)
    ones_sq = np.ones((DH, DH), adt)
    ones_ra = np.ones((1, DH), adt)

    in_maps = []
    for c in range(NCORES):
        b, hg = divmod(c, 2)
        e0 = hg * EH
        in_maps.append(
            {
                "qT": qTa[b],
                "kT": kTa[b],
                "wqT": np.ascontiguousarray(WqT[:, e0 : e0 + EH]),
                "wkT": np.ascontiguousarray(WkT[:, e0 : e0 + EH]),
                "wvT": np.ascontiguousarray(WvT[:, e0 : e0 + EH]),
                "bq_d": np.ascontiguousarray(bq[e0 : e0 + EH].reshape(HPC, DH).T),
                "bk_d": np.ascontiguousarray(bk[e0 : e0 + EH].reshape(HPC, DH).T),
                "bv_d": np.ascontiguousarray(
                    bv[e0 : e0 + EH].reshape(1, EH)
                ).astype(adt),
                "kbias_d": np.ascontiguousarray(kbias_all[b].reshape(NKT, DH).T),
                "tri_d": tri,
                "ones_sq_d": ones_sq,
                "ones_ra_d": ones_ra,
                "resid_d": np.ascontiguousarray(qTa[b][e0 : e0 + EH, :]),
            }
        )
    return in_maps


def _assemble(results):
    """results: list of 8 dicts with outT_d [EH, S] -> full [B, S, HID]."""
    out = np.empty((B, S, HID), np.float32)
    for c in range(NCORES):
        b, hg = divmod(c, 2)
        out[b, :, hg * EH : (hg + 1) * EH] = results[c]["outT_d"].T
    return out


def kernel(**inputs):
    from concourse.bass_utils import run_bass_kernel_spmd

    # fast path is valid unless some key row is exactly zero-sum (padding)
    keys = np.asarray(inputs["keys"], np.float32)
    fast = not bool(np.any(keys.sum(axis=-1) == 0.0))
    nc = _build(fast=fast, act_dt=ACT_DT)
    in_maps = _host_prep(**inputs, act_dt=ACT_DT)
    res = run_bass_kernel_spmd(nc, in_maps, core_ids=list(range(NCORES)))
    kernel.last_results = res
    return _assemble(res.results)
